# revision 22
# baseline (speedup 1.0000x reference)
"""GraphTransformerLayer on 8 Trainium2 NeuronCores (Bass/Tile).

Sharding: 8-way along the query-node axis. Each core owns NQ=512 query rows,
computes full K/V projections (replicated), its slice of masked attention,
and its slice of the FFN. No collectives; the host concatenates the slices.

v3:
- Q/K/V and Wo matmuls in fp8e4 DoubleRow perf mode (2 k-subtiles per
  instruction, 0.5 cycles/row); weights host-folded to [128, 4, .], x32
  scaled (ctx x64) for fp8 range. bk dropped (exact under softmax), bv
  folded into host bo' = bo + bv @ Wo.T.
- Wo streams per head-pair into an SBUF f32 accumulator; ctx/wo PSUM tiles
  share one 4-deep ring so normalization lag never stalls the next pair.
- All DMA'd tensors host-packed to their exact SBUF layouts (contiguous,
  hardware-DGE friendly); issued in first-use order, w1/w2 before attention.
- LayerNorms use batched [128,4] stats + rstd across the 4 query tiles,
  ACT-engine center/scale, f32 PE transposes (no bf16 staging copy).
- FFN1/FFN2 interleaved per ft tile; per-qt LN2 + output DMA.
"""

import sys

if "/opt/trn_rl_repo" not in sys.path:
    sys.path.insert(0, "/opt/trn_rl_repo")

import numpy as np
import ml_dtypes

import concourse.bacc as bacc
import concourse.tile as tile
import concourse.mybir as mybir
from concourse.bass_utils import run_bass_kernel_spmd

BF16 = ml_dtypes.bfloat16
FP8 = ml_dtypes.float8_e4m3
F32 = mybir.dt.float32
BF = mybir.dt.bfloat16
F8 = mybir.dt.float8e4

N = 4096
D = 512
H = 8
DK = 64
DFF = 2048
NCORES = 8
NQ = N // NCORES
P = 128
EPS = 1e-5
WS = 32.0  # host weight pre-scale for fp8
CS = 64.0  # ctx pre-scale for fp8

ALU = mybir.AluOpType
AF = mybir.ActivationFunctionType
DR = mybir.MatmulPerfMode.DoubleRow

# set by test.py to capture a profile
TRACE = False
TRACE_DIR = None
LAST_EXEC_NS = None

# debug: truncate after a phase (1=proj, 2=attention ctx, 3=h1acc, 4=full)
STOP_AT = 4

_CACHED = None


def _build():
    nc = bacc.Bacc("TRN2", target_bir_lowering=False, debug=False,
                   num_devices=NCORES)

    # folded fp8 tensors: [128, 4, C]; d = slot*128 + p
    hT2 = nc.dram_tensor("hT2", [P, 4, N], F8, kind="ExternalInput").ap()
    hqT2 = nc.dram_tensor("hqT2", [P, 4, NQ], F8, kind="ExternalInput").ap()
    wq2 = nc.dram_tensor("wq2", [P, 4, D], F8, kind="ExternalInput").ap()
    wk2 = nc.dram_tensor("wk2", [P, 4, D], F8, kind="ExternalInput").ap()
    wv2 = nc.dram_tensor("wv2", [P, 4, D], F8, kind="ExternalInput").ap()
    wo2 = nc.dram_tensor("wo2", [P, 4, D], F8, kind="ExternalInput").ap()
    maskP = nc.dram_tensor("maskP", [P, 32, NQ], BF, kind="ExternalInput").ap()
    w1P = nc.dram_tensor("w1P", [P, 4, DFF], BF, kind="ExternalInput").ap()
    w2P = nc.dram_tensor("w2P", [P, 16, D], BF, kind="ExternalInput").ap()
    bqP = nc.dram_tensor("bqP", [P, 4], F32, kind="ExternalInput").ap()
    b1P = nc.dram_tensor("b1P", [P, 16], F32, kind="ExternalInput").ap()
    b22 = nc.dram_tensor("b22", [1, D], BF, kind="ExternalInput").ap()
    lnc = nc.dram_tensor("lnc", [P, 4 * D], F32, kind="ExternalInput").ap()
    hqP = nc.dram_tensor("hqP", [P, 4, D], F32, kind="ExternalInput").ap()
    identf = nc.dram_tensor("identf", [P, P], F32, kind="ExternalInput").ap()
    out = nc.dram_tensor("out", [NQ, D], F32, kind="ExternalOutput").ap()

    with tile.TileContext(nc) as tc:
        _emit(nc, tc, locals())
    nc.compile()
    return nc


def _emit(nc, tc, t):
    hT2, hqT2, maskP = t["hT2"], t["hqT2"], t["maskP"]
    wq2, wk2, wv2, wo2 = t["wq2"], t["wk2"], t["wv2"], t["wo2"]
    w1P, w2P = t["w1P"], t["w2P"]
    bqP, b1P, b22, lnc, hqP = t["bqP"], t["b1P"], t["b22"], t["lnc"], t["hqP"]
    identf, out = t["identf"], t["out"]

    from contextlib import ExitStack

    es = ExitStack()
    with es:
        cpool = es.enter_context(tc.tile_pool(name="const", bufs=1))
        h1pool = es.enter_context(tc.tile_pool(name="h1p", bufs=1))
        qkv_es = ExitStack()
        qkvpool = qkv_es.enter_context(tc.tile_pool(name="qkvp", bufs=1))
        mpool = qkv_es.enter_context(tc.tile_pool(name="maskp", bufs=1))
        proj_es = ExitStack()
        ppool = proj_es.enter_context(tc.tile_pool(name="projp", bufs=1))

        # ---- DMAs, first-use order, all contiguous host-packed ----
        wq_sb = ppool.tile([P, 4, D], F8, tag="wq")
        nc.sync.dma_start(wq_sb[:], wq2[:])
        hqT_sb = ppool.tile([P, 4, NQ], F8, tag="hqT")
        nc.sync.dma_start(hqT_sb[:], hqT2[:])
        bq_sb = cpool.tile([P, 4], F32, tag="bq")
        nc.sync.dma_start(bq_sb[:], bqP[:])
        wk_sb = ppool.tile([P, 4, D], F8, tag="wk")
        nc.sync.dma_start(wk_sb[:], wk2[:])
        hT_sb = ppool.tile([P, 4, N], F8, tag="hT")
        for ck in range(4):
            nc.sync.dma_start(hT_sb[:, :, ck * (N // 4):(ck + 1) * (N // 4)],
                              hT2[:, :, ck * (N // 4):(ck + 1) * (N // 4)])
        wv_sb = ppool.tile([P, 4, D], F8, tag="wv")
        nc.sync.dma_start(wv_sb[:], wv2[:])
        # mask in SBUF layout [128, mt, NQ], two halves on the gpsimd queue.
        # A dummy gpsimd read of hT_sb delays the 4MB mask transfer until the
        # critical-path hT/weight DMAs have drained (shared HBM bandwidth).
        mask_sb = mpool.tile([P, 32, NQ], BF, tag="mask")
        gate = cpool.tile([1, 4], F8, tag="gate")
        nc.gpsimd.tensor_copy(gate[:], hT_sb[0:1, :, 0:1])
        nc.gpsimd.dma_start(mask_sb[:, 0:16, :], maskP[:, 0:16, :])
        nc.gpsimd.dma_start(mask_sb[:, 16:32, :], maskP[:, 16:32, :])
        wo_sb = cpool.tile([P, 4, D], F8, tag="wo")
        nc.sync.dma_start(wo_sb[:], wo2[:])
        lnab = cpool.tile([P, 4 * D], F32, tag="lnc")
        nc.sync.dma_start(lnab[:], lnc[:])
        hq_sb = cpool.tile([P, 4, D], F32, tag="hq")
        nc.sync.dma_start(hq_sb[:], hqP[:])
        b1_sb = cpool.tile([P, 16], F32, tag="b1")
        nc.sync.dma_start(b1_sb[:], b1P[:])
        b2_sb = cpool.tile([1, D], BF, tag="b2")
        nc.sync.dma_start(b2_sb[:], b22[:])
        identf_sb = cpool.tile([P, P], F32, tag="idf")
        nc.sync.dma_start(identf_sb[:], identf[:])
        ones_sb = cpool.tile([1, P], BF, tag="ones")
        nc.vector.memset(ones_sb[:], 1.0)
        eps_sb = cpool.tile([P, 1], F32, tag="eps")
        nc.vector.memset(eps_sb[:], EPS)

        g1l = lnab[:, 0:D]
        b1l = lnab[:, D:2 * D]
        g2l = lnab[:, 2 * D:3 * D]
        b2l = lnab[:, 3 * D:4 * D]

        # ---- persistent attention state ----
        kT_sb = [qkvpool.tile([P, N], BF, tag=f"kt{i}", name=f"kT{i}")
                 for i in range(4)]
        qT_sb = [qkvpool.tile([P, NQ], BF, tag=f"qt{i}", name=f"qT{i}")
                 for i in range(4)]
        v_sb = [qkvpool.tile([P, H * (DK + 1)], BF, tag=f"v{i}", name=f"v{i}")
                for i in range(32)]
        for mt in range(32):
            vv = v_sb[mt].rearrange("p (h c) -> p h c", c=DK + 1)
            nc.vector.memset(vv[:, :, DK:DK + 1], 1.0)
        ctxT2 = [h1pool.tile([P, 2 * NQ], F8, tag=f"cx{i}", name=f"ctxT2{i}")
                 for i in range(2)]
        h1acc = [h1pool.tile([P, D], F32, tag=f"ha{i}", name=f"h1acc{i}")
                 for i in range(4)]
        h1_sb = [h1pool.tile([P, D], F32, tag=f"h1_{i}", name=f"h1_{i}")
                 for i in range(4)]
        h1T_sb = [h1pool.tile([P, NQ], BF, tag=f"h1T{i}", name=f"h1T{i}")
                  for i in range(4)]

        # ================= projections (fp8 DoubleRow) =================
        with tc.tile_pool(name="psproj", bufs=4, space="PSUM") as psp:
            for tt in range(4):
                ps = psp.tile([P, NQ], F32, tag="pp", name="ps_q")
                for sp in range(2):
                    nc.tensor.matmul(ps[:],
                                     wq_sb[:, 2 * sp:2 * sp + 2,
                                           tt * P:(tt + 1) * P],
                                     hqT_sb[:, 2 * sp:2 * sp + 2, :],
                                     start=(sp == 0), stop=(sp == 1),
                                     perf_mode=DR)
                nc.scalar.activation(qT_sb[tt][:], ps[:], AF.Identity,
                                     bias=bq_sb[:, tt:tt + 1], scale=1.0 / WS)
            for tt in range(4):
                for c in range(8):
                    ps = psp.tile([P, D], F32, tag="pp", name="ps_k")
                    for sp in range(2):
                        nc.tensor.matmul(ps[:],
                                         wk_sb[:, 2 * sp:2 * sp + 2,
                                               tt * P:(tt + 1) * P],
                                         hT_sb[:, 2 * sp:2 * sp + 2,
                                               c * D:(c + 1) * D],
                                         start=(sp == 0), stop=(sp == 1),
                                         perf_mode=DR)
                    nc.vector.tensor_scalar_mul(
                        kT_sb[tt][:, c * D:(c + 1) * D], ps[:], 1.0 / WS)
            for mc in range(32):
                ps = psp.tile([P, D], F32, tag="pp", name="ps_v")
                for sp in range(2):
                    nc.tensor.matmul(ps[:],
                                     hT_sb[:, 2 * sp:2 * sp + 2,
                                           mc * P:(mc + 1) * P],
                                     wv_sb[:, 2 * sp:2 * sp + 2, :],
                                     start=(sp == 0), stop=(sp == 1),
                                     perf_mode=DR)
                vv = v_sb[mc].rearrange("p (h c) -> p h c", c=DK + 1)
                nc.scalar.activation(vv[:, :, 0:DK],
                                     ps.rearrange("p (h c) -> p h c", c=DK),
                                     AF.Copy, scale=1.0 / WS)
        proj_es.close()

        if STOP_AT == 1:
            for qt in range(4):
                cv = h1pool.tile([P, D], F32, tag="dbg", bufs=2, name="cv")
                nc.vector.tensor_add(cv[:], kT_sb[qt][:, 0:D], qT_sb[qt][:])
                nc.vector.tensor_add(cv[:], cv[:], v_sb[qt * 8][:, 0:D])
                nc.sync.dma_start(out[qt * P:(qt + 1) * P, :], cv[:])
            qkv_es.close()
            return

        # ================= attention =================
        with tc.tile_pool(name="attp", bufs=1) as apool, \
             tc.tile_pool(name="psatt", bufs=1, space="PSUM") as psa:
            def _norm_one(pend, i):
                # normalize head i of a completed pair from its SBUF eviction
                ctxe, dst, col = pend
                po = i * DK
                rec = apool.tile([1, NQ], F32, tag="rec", bufs=2, name="rec")
                nc.vector.reciprocal(
                    rec[:], ctxe[DK:DK + 1, i * NQ:(i + 1) * NQ])
                bc = apool.tile([P, NQ], F32, tag="bc", bufs=2, name="bc")
                nc.gpsimd.partition_broadcast(bc[:], rec[:])
                nc.vector.scalar_tensor_tensor(
                    dst[po:po + DK, col:col + NQ],
                    ctxe[0:DK, i * NQ:(i + 1) * NQ],
                    CS, bc[0:DK, :], op0=ALU.mult, op1=ALU.mult)

            pend = None
            for hp in range(4):
                ctx_ps = psa.tile([P, 2 * NQ], F32, tag="pc", bufs=1,
                                  name="ctx_ps")
                for g in range(16):
                    sp = [psa.tile([P, 1024], F32, tag="ps", bufs=3,
                                   name="sc_ps") for _ in range(2)]
                    at = [apool.tile([P, 1024], BF, tag="at", bufs=6,
                                     name="at") for _ in range(2)]
                    for i, po in ((0, 0), (1, DK)):
                        for j in range(2):
                            mt = 2 * g + j
                            nc.tensor.matmul(
                                sp[i][:, j * NQ:(j + 1) * NQ],
                                kT_sb[hp][po:po + DK, mt * P:(mt + 1) * P],
                                qT_sb[hp][po:po + DK, :],
                                start=True, stop=True)
                    for i in range(2):
                        nc.scalar.activation(at[i][:], sp[i][:], AF.Exp,
                                             scale=0.125)
                        nc.vector.tensor_mul(
                            at[i][:], at[i][:],
                            mask_sb[:, 2 * g:2 * g + 2, :])
                    for i, h in ((0, 2 * hp), (1, 2 * hp + 1)):
                        for j in range(2):
                            mt = 2 * g + j
                            nc.tensor.matmul(
                                ctx_ps[0:DK + 1, i * NQ:(i + 1) * NQ],
                                v_sb[mt][:, h * 65:h * 65 + 65],
                                at[i][:, j * NQ:(j + 1) * NQ],
                                start=(mt == 0), stop=(mt == 31))
                    # previous pair's normalization, deferred so its 3.3us
                    # reciprocals don't block this pair's mask muls on DVE
                    if pend is not None and g in (2, 6):
                        _norm_one(pend, 0 if g == 2 else 1)
                        if g == 6:
                            pend = None
                # evict ctx+rowsums to SBUF fast (frees the psum bank for
                # the next head-pair); normalization happens next pair
                ctxe = apool.tile([DK + 1, 2 * NQ], F32, tag="ce", bufs=2,
                                  name="ctxe")
                nc.vector.tensor_copy(ctxe[:], ctx_ps[0:DK + 1, :])
                pend = (ctxe, ctxT2[hp // 2], (hp % 2) * NQ)
            _norm_one(pend, 0)
            _norm_one(pend, 1)
            pend = None

        if STOP_AT == 2:
            for qt in range(4):
                cv = h1pool.tile([P, D], F32, tag="dbg", bufs=2, name="cv")
                nc.vector.tensor_copy(
                    cv[:], ctxT2[qt // 2][:, (qt % 2) * NQ:(qt % 2 + 1) * NQ])
                nc.sync.dma_start(out[qt * P:(qt + 1) * P, :], cv[:])
            qkv_es.close()
            return
        if STOP_AT == 3:
            for qt in range(4):
                nc.sync.dma_start(out[qt * P:(qt + 1) * P, :], h1acc[qt][:])
            qkv_es.close()
            return

        # ---- Wo + LN1 (batched stats) + f32 transpose ----
        src3 = [ctxT2[spx].rearrange("p (i n) -> p i n", n=NQ)
                for spx in range(2)]
        with tc.tile_pool(name="pspost", bufs=2, space="PSUM") as psw:
            s1 = h1pool.tile([P, 4], F32, tag="s1a", name="s1a")
            s2 = h1pool.tile([P, 4], F32, tag="s2a", name="s2a")
            for qt in range(4):
                wops = psw.tile([P, D], F32, tag="wo", bufs=2, name="wo_ps")
                for spx in range(2):
                    nc.tensor.matmul(wops[:],
                                     src3[spx][:, :, qt * P:(qt + 1) * P],
                                     wo_sb[:, 2 * spx:2 * spx + 2, :],
                                     start=(spx == 0), stop=(spx == 1),
                                     perf_mode=DR)
                nc.vector.scalar_tensor_tensor(
                    h1acc[qt][:], wops[:], 1.0 / (WS * CS),
                    hq_sb[:, qt:qt + 1, :], op0=ALU.mult, op1=ALU.add,
                    accum_out=s1[:, qt:qt + 1])
                xsq = h1pool.tile([P, D], F32, tag="xsq", bufs=4, name="xsq")
                eng = nc.vector if qt < 2 else nc.gpsimd
                eng.tensor_mul(xsq[:], h1acc[qt][:], h1acc[qt][:])
                nc.vector.reduce_sum(s2[:, qt:qt + 1], xsq[:],
                                     axis=mybir.AxisListType.X)
            rstd4, nmr4 = _stats4(nc, h1pool, s1, s2, eps_sb, "a")
            for qt in range(4):
                xn = h1pool.tile([P, D], F32, tag="xn", bufs=2, name="xn")
                nc.scalar.activation(xn[:], h1acc[qt][:], AF.Identity,
                                     bias=nmr4[:, qt:qt + 1],
                                     scale=rstd4[:, qt:qt + 1])
                eng = nc.vector if qt < 2 else nc.gpsimd
                eng.tensor_mul(h1_sb[qt][:], xn[:], g1l)
                eng.tensor_add(h1_sb[qt][:], h1_sb[qt][:], b1l)
                for i in range(4):
                    tp = psw.tile([P, P], F32, tag="tp", name="tp")
                    nc.tensor.transpose(tp[:], h1_sb[qt][:, i * P:(i + 1) * P],
                                        identf_sb[:])
                    nc.vector.tensor_copy(
                        h1T_sb[i][:, qt * P:(qt + 1) * P], tp[:])

        qkv_es.close()

        # ================= FFN (ft-interleaved) =================
        ffnp = es.enter_context(tc.tile_pool(name="ffnp", bufs=1))
        w1_sb = ffnp.tile([P, 4, DFF], BF, tag="w1")
        nc.sync.dma_start(w1_sb[:], w1P[:])
        w2_sb = ffnp.tile([P, 16, D], BF, tag="w2")
        nc.sync.dma_start(w2_sb[:], w2P[:])
        with tc.tile_pool(name="psffn", bufs=1, space="PSUM") as psf:
            ff_ps = [psf.tile([P, D], F32, tag=f"fa{i}", name=f"ff_ps{i}")
                     for i in range(4)]
            for ft in range(16):
                ps = psf.tile([P, NQ], F32, tag="pf", bufs=2, name="f_ps")
                for s in range(4):
                    nc.tensor.matmul(ps[:],
                                     w1_sb[:, s:s + 1, ft * P:(ft + 1) * P],
                                     h1T_sb[s][:], start=(s == 0),
                                     stop=(s == 3))
                fT = ffnp.tile([P, NQ], BF, tag="fT", bufs=3, name="fT")
                nc.scalar.activation(fT[:], ps[:], AF.Relu,
                                     bias=b1_sb[:, ft:ft + 1])
                for qt in range(4):
                    nc.tensor.matmul(ff_ps[qt][:],
                                     fT[:, qt * P:(qt + 1) * P],
                                     w2_sb[:, ft:ft + 1, :], start=(ft == 0),
                                     stop=False)
            # ---- +b2, then LN2 with batched stats ----
            s1 = h1pool.tile([P, 4], F32, tag="s1b", name="s1b")
            s2 = h1pool.tile([P, 4], F32, tag="s2b", name="s2b")
            x2 = []
            for qt in range(4):
                nc.tensor.matmul(ff_ps[qt][:], ones_sb[:], b2_sb[:],
                                 start=False, stop=True)
                x = h1pool.tile([P, D], F32, tag=f"x2{qt}", name=f"x2{qt}")
                nc.vector.scalar_tensor_tensor(x[:], ff_ps[qt][:], 0.0,
                                               h1_sb[qt][:], op0=ALU.add,
                                               op1=ALU.add,
                                               accum_out=s1[:, qt:qt + 1])
                xsq = h1pool.tile([P, D], F32, tag="xsq", bufs=4, name="xsq")
                eng = nc.vector if qt < 2 else nc.gpsimd
                eng.tensor_mul(xsq[:], x[:], x[:])
                nc.vector.reduce_sum(s2[:, qt:qt + 1], xsq[:],
                                     axis=mybir.AxisListType.X)
                x2.append(x)
            rstd4, nmr4 = _stats4(nc, h1pool, s1, s2, eps_sb, "b")
            for qt in range(4):
                xn = h1pool.tile([P, D], F32, tag="xn", bufs=2, name="xn")
                nc.scalar.activation(xn[:], x2[qt][:], AF.Identity,
                                     bias=nmr4[:, qt:qt + 1],
                                     scale=rstd4[:, qt:qt + 1])
                h2 = h1pool.tile([P, D], F32, tag="h2o", bufs=4, name="h2")
                eng = nc.vector if qt < 2 else nc.gpsimd
                eng.tensor_mul(h2[:], xn[:], g2l)
                eng.tensor_add(h2[:], h2[:], b2l)
                nc.sync.dma_start(out[qt * P:(qt + 1) * P, :], h2[:])


def _stats4(nc, pool, s1, s2, eps_sb, uid):
    """Batched LN stats: from per-qt sums s1,s2 [P,4] compute rstd4 and
    nmr4 = (-mean * rstd) [P,4]."""
    I32 = mybir.dt.int32
    nm = pool.tile([P, 4], F32, tag="nm4", bufs=2, name=f"nm4{uid}")
    nc.vector.tensor_scalar_mul(nm[:], s1[:], -1.0 / D)
    m2 = pool.tile([P, 4], F32, tag="m24", bufs=2, name=f"m24{uid}")
    nc.vector.tensor_mul(m2[:], nm[:], nm[:])
    var = pool.tile([P, 4], F32, tag="var4", bufs=2, name=f"var4{uid}")
    nc.vector.scalar_tensor_tensor(var[:], s2[:], 1.0 / D, m2[:],
                                   op0=ALU.mult, op1=ALU.subtract)
    ve = pool.tile([P, 4], F32, tag="ve4", bufs=2, name=f"ve4{uid}")
    nc.vector.tensor_scalar_add(ve[:], var[:], eps_sb[:])
    rstd = pool.tile([P, 4], F32, tag="rs4", bufs=2, name=f"rs4{uid}")
    nc.vector.tensor_single_scalar(rstd[:].bitcast(I32), ve[:].bitcast(I32),
                                   1, op=ALU.arith_shift_right)
    nc.vector.tensor_single_scalar(rstd[:].bitcast(I32), rstd[:].bitcast(I32),
                                   0x5F3759DF, op=ALU.subtract)
    nc.vector.tensor_single_scalar(rstd[:].bitcast(I32), rstd[:].bitcast(I32),
                                   -1, op=ALU.mult)
    tq = pool.tile([P, 4], F32, tag="tq4", bufs=2, name=f"tq4{uid}")
    for _ in range(3):
        nc.vector.tensor_mul(tq[:], rstd[:], rstd[:])
        nc.vector.tensor_mul(tq[:], tq[:], ve[:])
        nc.vector.tensor_scalar_mul(tq[:], tq[:], -0.5)
        nc.vector.tensor_scalar_add(tq[:], tq[:], 1.5)
        nc.vector.tensor_mul(rstd[:], rstd[:], tq[:])
    nmr = pool.tile([P, 4], F32, tag="nmr4", bufs=2, name=f"nmr4{uid}")
    nc.vector.tensor_mul(nmr[:], nm[:], rstd[:])
    return rstd, nmr


def _fold(xT):
    """[512, C] -> [128, 4, C] with d = slot*128 + p."""
    c = xT.shape[1]
    return np.ascontiguousarray(xT.reshape(4, P, c).transpose(1, 0, 2))


def _prep_inputs(inputs):
    f32 = np.float32
    h = np.asarray(inputs["h"], f32)
    adj = np.asarray(inputs["adj"])

    def bf(x):
        return np.ascontiguousarray(np.asarray(x, f32).astype(BF16))

    def f8(x):
        return np.ascontiguousarray(np.asarray(x, f32).astype(FP8))

    hT = np.ascontiguousarray(h.T)
    adjb = (adj != 0)
    np.fill_diagonal(adjb, True)
    adjb_bf = adjb.astype(BF16)

    wq, wk, wv, wo = (np.asarray(inputs[k], f32)
                      for k in ("Wq", "Wk", "Wv", "Wo"))
    w1, w2 = np.asarray(inputs["W1"], f32), np.asarray(inputs["W2"], f32)
    bv = np.asarray(inputs["bv"], f32)
    bo = np.asarray(inputs["bo"], f32)
    bo2 = bo + bv @ wo.T  # bv folded through Wo

    lnc = np.concatenate([
        np.broadcast_to(np.asarray(inputs[k], f32), (P, D))
        for k in ("ln1_g", "ln1_b", "ln2_g", "ln2_b")], axis=1)

    shared = {
        "hT2": f8(_fold(hT)),
        "wq2": f8(_fold(wq.T) * WS), "wk2": f8(_fold(wk.T) * WS),
        "wv2": f8(_fold(wv.T) * WS), "wo2": f8(_fold(wo.T) * WS),
        "w1P": bf(w1.T.reshape(4, P, DFF).transpose(1, 0, 2)),
        "w2P": bf(w2.T.reshape(16, P, D).transpose(1, 0, 2)),
        "bqP": np.ascontiguousarray(
            np.asarray(inputs["bq"], f32).reshape(4, P).T),
        "b1P": np.ascontiguousarray(
            np.asarray(inputs["b1"], f32).reshape(16, P).T),
        "b22": bf(np.asarray(inputs["b2"], f32)[None, :]),
        "lnc": np.ascontiguousarray(lnc),
        "identf": np.eye(P, dtype=f32),
    }
    in_maps = []
    for i in range(NCORES):
        r0 = i * NQ
        m = dict(shared)
        m["hqT2"] = f8(_fold(np.ascontiguousarray(hT[:, r0:r0 + NQ])))
        m["hqP"] = np.ascontiguousarray(
            (h[r0:r0 + NQ, :] + bo2).reshape(4, P, D).transpose(1, 0, 2))
        m["maskP"] = np.ascontiguousarray(
            adjb_bf[r0:r0 + NQ, :].T.reshape(32, P, NQ).transpose(1, 0, 2))
        in_maps.append(m)
    return in_maps


def kernel(**inputs) -> np.ndarray:
    global _CACHED, LAST_EXEC_NS
    if _CACHED is None:
        _CACHED = _build()
    nc = _CACHED
    in_maps = _prep_inputs(inputs)
    kw = {}
    if TRACE:
        kw = dict(trace=True, tmpdir=TRACE_DIR)
    res = run_bass_kernel_spmd(nc, in_maps, list(range(NCORES)), **kw)
    LAST_EXEC_NS = res.exec_time_ns
    return np.concatenate([res.results[i]["out"] for i in range(NCORES)],
                          axis=0)


# revision 23
# speedup vs baseline: 1.1546x; 1.1546x over previous
"""GraphTransformerLayer on 8 Trainium2 NeuronCores (Bass/Tile).

Sharding: 8-way along the query-node axis. Each core owns NQ=512 query rows,
computes full K/V projections (replicated), its slice of masked attention,
and its slice of the FFN. No collectives; the host concatenates the slices.

v3:
- Q/K/V and Wo matmuls in fp8e4 DoubleRow perf mode (2 k-subtiles per
  instruction, 0.5 cycles/row); weights host-folded to [128, 4, .], x32
  scaled (ctx x64) for fp8 range. bk dropped (exact under softmax), bv
  folded into host bo' = bo + bv @ Wo.T.
- Wo streams per head-pair into an SBUF f32 accumulator; ctx/wo PSUM tiles
  share one 4-deep ring so normalization lag never stalls the next pair.
- All DMA'd tensors host-packed to their exact SBUF layouts (contiguous,
  hardware-DGE friendly); issued in first-use order, w1/w2 before attention.
- LayerNorms use batched [128,4] stats + rstd across the 4 query tiles,
  ACT-engine center/scale, f32 PE transposes (no bf16 staging copy).
- FFN1/FFN2 interleaved per ft tile; per-qt LN2 + output DMA.
"""

import sys

if "/opt/trn_rl_repo" not in sys.path:
    sys.path.insert(0, "/opt/trn_rl_repo")

import numpy as np
import ml_dtypes

import concourse.bacc as bacc
import concourse.tile as tile
import concourse.mybir as mybir
from concourse.bass_utils import run_bass_kernel_spmd

BF16 = ml_dtypes.bfloat16
FP8 = ml_dtypes.float8_e4m3
F32 = mybir.dt.float32
BF = mybir.dt.bfloat16
F8 = mybir.dt.float8e4

N = 4096
D = 512
H = 8
DK = 64
DFF = 2048
NCORES = 8
NQ = N // NCORES
P = 128
EPS = 1e-5
WS = 32.0  # host weight pre-scale for fp8
CS = 64.0  # ctx pre-scale for fp8

ALU = mybir.AluOpType
AF = mybir.ActivationFunctionType
DR = mybir.MatmulPerfMode.DoubleRow

# set by test.py to capture a profile
TRACE = False
TRACE_DIR = None
LAST_EXEC_NS = None

# debug: truncate after a phase (1=proj, 2=attention ctx, 3=h1acc, 4=full)
STOP_AT = 4

_CACHED = None


def _build():
    nc = bacc.Bacc("TRN2", target_bir_lowering=False, debug=False,
                   num_devices=NCORES)

    # folded fp8 tensors: [128, 4, C]; d = slot*128 + p
    hT2 = nc.dram_tensor("hT2", [P, 4, N], F8, kind="ExternalInput").ap()
    hqT2 = nc.dram_tensor("hqT2", [P, 4, NQ], F8, kind="ExternalInput").ap()
    wq2 = nc.dram_tensor("wq2", [P, 4, D], F8, kind="ExternalInput").ap()
    wk2 = nc.dram_tensor("wk2", [P, 4, D], F8, kind="ExternalInput").ap()
    wv2 = nc.dram_tensor("wv2", [P, 4, D], F8, kind="ExternalInput").ap()
    wo2 = nc.dram_tensor("wo2", [P, 4, D], F8, kind="ExternalInput").ap()
    maskP = nc.dram_tensor("maskP", [P, 32, NQ], BF, kind="ExternalInput").ap()
    w1P = nc.dram_tensor("w1P", [P, 4, DFF], BF, kind="ExternalInput").ap()
    w2P = nc.dram_tensor("w2P", [P, 16, D], BF, kind="ExternalInput").ap()
    bqP = nc.dram_tensor("bqP", [P, 4], F32, kind="ExternalInput").ap()
    b1P = nc.dram_tensor("b1P", [P, 16], F32, kind="ExternalInput").ap()
    b22 = nc.dram_tensor("b22", [1, D], BF, kind="ExternalInput").ap()
    lnc = nc.dram_tensor("lnc", [P, 4 * D], F32, kind="ExternalInput").ap()
    hqP = nc.dram_tensor("hqP", [P, 4, D], F32, kind="ExternalInput").ap()
    identf = nc.dram_tensor("identf", [P, P], F32, kind="ExternalInput").ap()
    out = nc.dram_tensor("out", [NQ, D], F32, kind="ExternalOutput").ap()

    with tile.TileContext(nc) as tc:
        _emit(nc, tc, locals())
    nc.compile()
    return nc


def _emit(nc, tc, t):
    hT2, hqT2, maskP = t["hT2"], t["hqT2"], t["maskP"]
    wq2, wk2, wv2, wo2 = t["wq2"], t["wk2"], t["wv2"], t["wo2"]
    w1P, w2P = t["w1P"], t["w2P"]
    bqP, b1P, b22, lnc, hqP = t["bqP"], t["b1P"], t["b22"], t["lnc"], t["hqP"]
    identf, out = t["identf"], t["out"]

    from contextlib import ExitStack

    es = ExitStack()
    with es:
        cpool = es.enter_context(tc.tile_pool(name="const", bufs=1))
        h1pool = es.enter_context(tc.tile_pool(name="h1p", bufs=1))
        qkv_es = ExitStack()
        qkvpool = qkv_es.enter_context(tc.tile_pool(name="qkvp", bufs=1))
        mpool = qkv_es.enter_context(tc.tile_pool(name="maskp", bufs=1))
        proj_es = ExitStack()
        ppool = proj_es.enter_context(tc.tile_pool(name="projp", bufs=1))

        # ---- DMAs, first-use order, all contiguous host-packed ----
        wq_sb = ppool.tile([P, 4, D], F8, tag="wq")
        nc.sync.dma_start(wq_sb[:], wq2[:])
        hqT_sb = ppool.tile([P, 4, NQ], F8, tag="hqT")
        nc.sync.dma_start(hqT_sb[:], hqT2[:])
        bq_sb = cpool.tile([P, 4], F32, tag="bq")
        nc.sync.dma_start(bq_sb[:], bqP[:])
        wk_sb = ppool.tile([P, 4, D], F8, tag="wk")
        nc.sync.dma_start(wk_sb[:], wk2[:])
        hT_sb = ppool.tile([P, 4, N], F8, tag="hT")
        for ck in range(4):
            nc.sync.dma_start(hT_sb[:, :, ck * (N // 4):(ck + 1) * (N // 4)],
                              hT2[:, :, ck * (N // 4):(ck + 1) * (N // 4)])
        wv_sb = ppool.tile([P, 4, D], F8, tag="wv")
        nc.sync.dma_start(wv_sb[:], wv2[:])
        # mask in SBUF layout [128, mt, NQ], two halves on the gpsimd queue.
        # A dummy gpsimd read of hT_sb delays the 4MB mask transfer until the
        # critical-path hT/weight DMAs have drained (shared HBM bandwidth).
        mask_sb = mpool.tile([P, 32, NQ], BF, tag="mask")
        gate = cpool.tile([1, 4], F8, tag="gate")
        nc.gpsimd.tensor_copy(gate[:], hT_sb[0:1, :, 0:1])
        nc.gpsimd.dma_start(mask_sb[:, 0:16, :], maskP[:, 0:16, :])
        nc.gpsimd.dma_start(mask_sb[:, 16:32, :], maskP[:, 16:32, :])
        wo_sb = cpool.tile([P, 4, D], F8, tag="wo")
        nc.sync.dma_start(wo_sb[:], wo2[:])
        lnab = cpool.tile([P, 4 * D], F32, tag="lnc")
        nc.sync.dma_start(lnab[:], lnc[:])
        hq_sb = cpool.tile([P, 4, D], F32, tag="hq")
        nc.sync.dma_start(hq_sb[:], hqP[:])
        b1_sb = cpool.tile([P, 16], F32, tag="b1")
        nc.sync.dma_start(b1_sb[:], b1P[:])
        b2_sb = cpool.tile([1, D], BF, tag="b2")
        nc.sync.dma_start(b2_sb[:], b22[:])
        identf_sb = cpool.tile([P, P], F32, tag="idf")
        nc.sync.dma_start(identf_sb[:], identf[:])
        ones_sb = cpool.tile([1, P], BF, tag="ones")
        nc.vector.memset(ones_sb[:], 1.0)
        eps_sb = cpool.tile([P, 1], F32, tag="eps")
        nc.vector.memset(eps_sb[:], EPS)

        g1l = lnab[:, 0:D]
        b1l = lnab[:, D:2 * D]
        g2l = lnab[:, 2 * D:3 * D]
        b2l = lnab[:, 3 * D:4 * D]

        # ---- persistent attention state ----
        kT_sb = [qkvpool.tile([P, N], BF, tag=f"kt{i}", name=f"kT{i}")
                 for i in range(4)]
        qT_sb = [qkvpool.tile([P, NQ], BF, tag=f"qt{i}", name=f"qT{i}")
                 for i in range(4)]
        v_sb = [qkvpool.tile([P, H * (DK + 1)], BF, tag=f"v{i}", name=f"v{i}")
                for i in range(32)]
        for mt in range(32):
            vv = v_sb[mt].rearrange("p (h c) -> p h c", c=DK + 1)
            nc.vector.memset(vv[:, :, DK:DK + 1], 1.0)
        ctxT2 = [h1pool.tile([P, 2 * NQ], F8, tag=f"cx{i}", name=f"ctxT2{i}")
                 for i in range(2)]
        h1acc = [h1pool.tile([P, D], F32, tag=f"ha{i}", name=f"h1acc{i}")
                 for i in range(4)]
        h1_sb = [h1pool.tile([P, D], F32, tag=f"h1_{i}", name=f"h1_{i}")
                 for i in range(4)]
        h1T_sb = [h1pool.tile([P, NQ], BF, tag=f"h1T{i}", name=f"h1T{i}")
                  for i in range(4)]

        # ================= projections (fp8 DoubleRow) =================
        with tc.tile_pool(name="psproj", bufs=4, space="PSUM") as psp:
            for tt in range(4):
                ps = psp.tile([P, NQ], F32, tag="pp", name="ps_q")
                for sp in range(2):
                    nc.tensor.matmul(ps[:],
                                     wq_sb[:, 2 * sp:2 * sp + 2,
                                           tt * P:(tt + 1) * P],
                                     hqT_sb[:, 2 * sp:2 * sp + 2, :],
                                     start=(sp == 0), stop=(sp == 1),
                                     perf_mode=DR)
                nc.scalar.activation(qT_sb[tt][:], ps[:], AF.Identity,
                                     bias=bq_sb[:, tt:tt + 1], scale=1.0 / WS)
            for tt in range(4):
                for c in range(8):
                    ps = psp.tile([P, D], F32, tag="pp", name="ps_k")
                    for sp in range(2):
                        nc.tensor.matmul(ps[:],
                                         wk_sb[:, 2 * sp:2 * sp + 2,
                                               tt * P:(tt + 1) * P],
                                         hT_sb[:, 2 * sp:2 * sp + 2,
                                               c * D:(c + 1) * D],
                                         start=(sp == 0), stop=(sp == 1),
                                         perf_mode=DR)
                    nc.vector.tensor_scalar_mul(
                        kT_sb[tt][:, c * D:(c + 1) * D], ps[:], 1.0 / WS)
            for mc in range(32):
                ps = psp.tile([P, D], F32, tag="pp", name="ps_v")
                for sp in range(2):
                    nc.tensor.matmul(ps[:],
                                     hT_sb[:, 2 * sp:2 * sp + 2,
                                           mc * P:(mc + 1) * P],
                                     wv_sb[:, 2 * sp:2 * sp + 2, :],
                                     start=(sp == 0), stop=(sp == 1),
                                     perf_mode=DR)
                vv = v_sb[mc].rearrange("p (h c) -> p h c", c=DK + 1)
                nc.scalar.activation(vv[:, :, 0:DK],
                                     ps.rearrange("p (h c) -> p h c", c=DK),
                                     AF.Copy, scale=1.0 / WS)
        proj_es.close()

        if STOP_AT == 1:
            for qt in range(4):
                cv = h1pool.tile([P, D], F32, tag="dbg", bufs=2, name="cv")
                nc.vector.tensor_add(cv[:], kT_sb[qt][:, 0:D], qT_sb[qt][:])
                nc.vector.tensor_add(cv[:], cv[:], v_sb[qt * 8][:, 0:D])
                nc.sync.dma_start(out[qt * P:(qt + 1) * P, :], cv[:])
            qkv_es.close()
            return

        # ================= attention =================
        with tc.tile_pool(name="attp", bufs=1) as apool, \
             tc.tile_pool(name="psatt", bufs=1, space="PSUM") as psa:
            for hp in range(4):
                ctx_ps = psa.tile([P, 2 * NQ], F32, tag="pc", bufs=1,
                                  name="ctx_ps")
                for g in range(16):
                    sp = [psa.tile([P, 1024], F32, tag="ps", bufs=3,
                                   name="sc_ps") for _ in range(2)]
                    at = [apool.tile([P, 1024], BF, tag="at", bufs=6,
                                     name="at") for _ in range(2)]
                    for i, po in ((0, 0), (1, DK)):
                        for j in range(2):
                            mt = 2 * g + j
                            nc.tensor.matmul(
                                sp[i][:, j * NQ:(j + 1) * NQ],
                                kT_sb[hp][po:po + DK, mt * P:(mt + 1) * P],
                                qT_sb[hp][po:po + DK, :],
                                start=True, stop=True)
                    for i in range(2):
                        nc.scalar.activation(at[i][:], sp[i][:], AF.Exp,
                                             scale=0.125)
                        nc.vector.tensor_mul(
                            at[i][:], at[i][:],
                            mask_sb[:, 2 * g:2 * g + 2, :])
                    for i, h in ((0, 2 * hp), (1, 2 * hp + 1)):
                        for j in range(2):
                            mt = 2 * g + j
                            nc.tensor.matmul(
                                ctx_ps[0:DK + 1, i * NQ:(i + 1) * NQ],
                                v_sb[mt][:, h * 65:h * 65 + 65],
                                at[i][:, j * NQ:(j + 1) * NQ],
                                start=(mt == 0), stop=(mt == 31))
                # evict ctx+rowsums to SBUF fast (frees the psum bank for
                # the next head-pair), then normalize from SBUF
                ctxe = apool.tile([DK + 1, 2 * NQ], F32, tag="ce", bufs=2,
                                  name="ctxe")
                nc.vector.tensor_copy(ctxe[:], ctx_ps[0:DK + 1, :])
                dst = ctxT2[hp // 2]
                col = (hp % 2) * NQ
                for i, po in ((0, 0), (1, DK)):
                    rec = apool.tile([1, NQ], F32, tag="rec", bufs=2,
                                     name="rec")
                    nc.vector.reciprocal(
                        rec[:], ctxe[DK:DK + 1, i * NQ:(i + 1) * NQ])
                    bc = apool.tile([P, NQ], F32, tag="bc", bufs=2, name="bc")
                    nc.gpsimd.partition_broadcast(bc[:], rec[:])
                    nc.vector.scalar_tensor_tensor(
                        dst[po:po + DK, col:col + NQ],
                        ctxe[0:DK, i * NQ:(i + 1) * NQ],
                        CS, bc[0:DK, :], op0=ALU.mult, op1=ALU.mult)
                if STOP_AT == 2:
                    continue

        if STOP_AT == 2:
            for qt in range(4):
                cv = h1pool.tile([P, D], F32, tag="dbg", bufs=2, name="cv")
                nc.vector.tensor_copy(
                    cv[:], ctxT2[qt // 2][:, (qt % 2) * NQ:(qt % 2 + 1) * NQ])
                nc.sync.dma_start(out[qt * P:(qt + 1) * P, :], cv[:])
            qkv_es.close()
            return
        if STOP_AT == 3:
            for qt in range(4):
                nc.sync.dma_start(out[qt * P:(qt + 1) * P, :], h1acc[qt][:])
            qkv_es.close()
            return

        # ---- Wo + LN1 (batched stats) + f32 transpose ----
        src3 = [ctxT2[spx].rearrange("p (i n) -> p i n", n=NQ)
                for spx in range(2)]
        with tc.tile_pool(name="pspost", bufs=2, space="PSUM") as psw:
            s1 = h1pool.tile([P, 4], F32, tag="s1a", name="s1a")
            s2 = h1pool.tile([P, 4], F32, tag="s2a", name="s2a")
            for qt in range(4):
                wops = psw.tile([P, D], F32, tag="wo", bufs=2, name="wo_ps")
                for spx in range(2):
                    nc.tensor.matmul(wops[:],
                                     src3[spx][:, :, qt * P:(qt + 1) * P],
                                     wo_sb[:, 2 * spx:2 * spx + 2, :],
                                     start=(spx == 0), stop=(spx == 1),
                                     perf_mode=DR)
                nc.vector.scalar_tensor_tensor(
                    h1acc[qt][:], wops[:], 1.0 / (WS * CS),
                    hq_sb[:, qt:qt + 1, :], op0=ALU.mult, op1=ALU.add,
                    accum_out=s1[:, qt:qt + 1])
                xsq = h1pool.tile([P, D], F32, tag="xsq", bufs=4, name="xsq")
                eng = nc.vector if qt < 2 else nc.gpsimd
                eng.tensor_mul(xsq[:], h1acc[qt][:], h1acc[qt][:])
                nc.vector.reduce_sum(s2[:, qt:qt + 1], xsq[:],
                                     axis=mybir.AxisListType.X)
            rstd4, nmr4 = _stats4(nc, h1pool, s1, s2, eps_sb, "a")
            for qt in range(4):
                xn = h1pool.tile([P, D], F32, tag="xn", bufs=2, name="xn")
                nc.scalar.activation(xn[:], h1acc[qt][:], AF.Identity,
                                     bias=nmr4[:, qt:qt + 1],
                                     scale=rstd4[:, qt:qt + 1])
                eng = nc.vector if qt < 2 else nc.gpsimd
                eng.tensor_mul(h1_sb[qt][:], xn[:], g1l)
                eng.tensor_add(h1_sb[qt][:], h1_sb[qt][:], b1l)
                for i in range(4):
                    tp = psw.tile([P, P], F32, tag="tp", name="tp")
                    nc.tensor.transpose(tp[:], h1_sb[qt][:, i * P:(i + 1) * P],
                                        identf_sb[:])
                    nc.vector.tensor_copy(
                        h1T_sb[i][:, qt * P:(qt + 1) * P], tp[:])

        qkv_es.close()

        # ================= FFN (ft-interleaved) =================
        ffnp = es.enter_context(tc.tile_pool(name="ffnp", bufs=1))
        w1_sb = ffnp.tile([P, 4, DFF], BF, tag="w1")
        nc.sync.dma_start(w1_sb[:], w1P[:])
        w2_sb = ffnp.tile([P, 16, D], BF, tag="w2")
        nc.sync.dma_start(w2_sb[:], w2P[:])
        with tc.tile_pool(name="psffn", bufs=1, space="PSUM") as psf:
            ff_ps = [psf.tile([P, D], F32, tag=f"fa{i}", name=f"ff_ps{i}")
                     for i in range(4)]
            for ft in range(16):
                ps = psf.tile([P, NQ], F32, tag="pf", bufs=2, name="f_ps")
                for s in range(4):
                    nc.tensor.matmul(ps[:],
                                     w1_sb[:, s:s + 1, ft * P:(ft + 1) * P],
                                     h1T_sb[s][:], start=(s == 0),
                                     stop=(s == 3))
                fT = ffnp.tile([P, NQ], BF, tag="fT", bufs=3, name="fT")
                nc.scalar.activation(fT[:], ps[:], AF.Relu,
                                     bias=b1_sb[:, ft:ft + 1])
                for qt in range(4):
                    nc.tensor.matmul(ff_ps[qt][:],
                                     fT[:, qt * P:(qt + 1) * P],
                                     w2_sb[:, ft:ft + 1, :], start=(ft == 0),
                                     stop=False)
            # ---- +b2, then LN2 with batched stats ----
            s1 = h1pool.tile([P, 4], F32, tag="s1b", name="s1b")
            s2 = h1pool.tile([P, 4], F32, tag="s2b", name="s2b")
            x2 = []
            for qt in range(4):
                nc.tensor.matmul(ff_ps[qt][:], ones_sb[:], b2_sb[:],
                                 start=False, stop=True)
                x = h1pool.tile([P, D], F32, tag=f"x2{qt}", name=f"x2{qt}")
                nc.vector.scalar_tensor_tensor(x[:], ff_ps[qt][:], 0.0,
                                               h1_sb[qt][:], op0=ALU.add,
                                               op1=ALU.add,
                                               accum_out=s1[:, qt:qt + 1])
                xsq = h1pool.tile([P, D], F32, tag="xsq", bufs=4, name="xsq")
                eng = nc.vector if qt < 2 else nc.gpsimd
                eng.tensor_mul(xsq[:], x[:], x[:])
                nc.vector.reduce_sum(s2[:, qt:qt + 1], xsq[:],
                                     axis=mybir.AxisListType.X)
                x2.append(x)
            rstd4, nmr4 = _stats4(nc, h1pool, s1, s2, eps_sb, "b")
            for qt in range(4):
                xn = h1pool.tile([P, D], F32, tag="xn", bufs=2, name="xn")
                nc.scalar.activation(xn[:], x2[qt][:], AF.Identity,
                                     bias=nmr4[:, qt:qt + 1],
                                     scale=rstd4[:, qt:qt + 1])
                h2 = h1pool.tile([P, D], F32, tag="h2o", bufs=4, name="h2")
                eng = nc.vector if qt < 2 else nc.gpsimd
                eng.tensor_mul(h2[:], xn[:], g2l)
                eng.tensor_add(h2[:], h2[:], b2l)
                nc.sync.dma_start(out[qt * P:(qt + 1) * P, :], h2[:])


def _stats4(nc, pool, s1, s2, eps_sb, uid):
    """Batched LN stats: from per-qt sums s1,s2 [P,4] compute rstd4 and
    nmr4 = (-mean * rstd) [P,4]."""
    I32 = mybir.dt.int32
    nm = pool.tile([P, 4], F32, tag="nm4", bufs=2, name=f"nm4{uid}")
    nc.vector.tensor_scalar_mul(nm[:], s1[:], -1.0 / D)
    m2 = pool.tile([P, 4], F32, tag="m24", bufs=2, name=f"m24{uid}")
    nc.vector.tensor_mul(m2[:], nm[:], nm[:])
    var = pool.tile([P, 4], F32, tag="var4", bufs=2, name=f"var4{uid}")
    nc.vector.scalar_tensor_tensor(var[:], s2[:], 1.0 / D, m2[:],
                                   op0=ALU.mult, op1=ALU.subtract)
    ve = pool.tile([P, 4], F32, tag="ve4", bufs=2, name=f"ve4{uid}")
    nc.vector.tensor_scalar_add(ve[:], var[:], eps_sb[:])
    rstd = pool.tile([P, 4], F32, tag="rs4", bufs=2, name=f"rs4{uid}")
    nc.vector.tensor_single_scalar(rstd[:].bitcast(I32), ve[:].bitcast(I32),
                                   1, op=ALU.arith_shift_right)
    nc.vector.tensor_single_scalar(rstd[:].bitcast(I32), rstd[:].bitcast(I32),
                                   0x5F3759DF, op=ALU.subtract)
    nc.vector.tensor_single_scalar(rstd[:].bitcast(I32), rstd[:].bitcast(I32),
                                   -1, op=ALU.mult)
    tq = pool.tile([P, 4], F32, tag="tq4", bufs=2, name=f"tq4{uid}")
    for _ in range(3):
        nc.vector.tensor_mul(tq[:], rstd[:], rstd[:])
        nc.vector.tensor_mul(tq[:], tq[:], ve[:])
        nc.vector.tensor_scalar_mul(tq[:], tq[:], -0.5)
        nc.vector.tensor_scalar_add(tq[:], tq[:], 1.5)
        nc.vector.tensor_mul(rstd[:], rstd[:], tq[:])
    nmr = pool.tile([P, 4], F32, tag="nmr4", bufs=2, name=f"nmr4{uid}")
    nc.vector.tensor_mul(nmr[:], nm[:], rstd[:])
    return rstd, nmr


def _fold(xT):
    """[512, C] -> [128, 4, C] with d = slot*128 + p."""
    c = xT.shape[1]
    return np.ascontiguousarray(xT.reshape(4, P, c).transpose(1, 0, 2))


def _prep_inputs(inputs):
    f32 = np.float32
    h = np.asarray(inputs["h"], f32)
    adj = np.asarray(inputs["adj"])

    def bf(x):
        return np.ascontiguousarray(np.asarray(x, f32).astype(BF16))

    def f8(x):
        return np.ascontiguousarray(np.asarray(x, f32).astype(FP8))

    hT = np.ascontiguousarray(h.T)
    adjb = (adj != 0)
    np.fill_diagonal(adjb, True)
    adjb_bf = adjb.astype(BF16)

    wq, wk, wv, wo = (np.asarray(inputs[k], f32)
                      for k in ("Wq", "Wk", "Wv", "Wo"))
    w1, w2 = np.asarray(inputs["W1"], f32), np.asarray(inputs["W2"], f32)
    bv = np.asarray(inputs["bv"], f32)
    bo = np.asarray(inputs["bo"], f32)
    bo2 = bo + bv @ wo.T  # bv folded through Wo

    lnc = np.concatenate([
        np.broadcast_to(np.asarray(inputs[k], f32), (P, D))
        for k in ("ln1_g", "ln1_b", "ln2_g", "ln2_b")], axis=1)

    shared = {
        "hT2": f8(_fold(hT)),
        "wq2": f8(_fold(wq.T) * WS), "wk2": f8(_fold(wk.T) * WS),
        "wv2": f8(_fold(wv.T) * WS), "wo2": f8(_fold(wo.T) * WS),
        "w1P": bf(w1.T.reshape(4, P, DFF).transpose(1, 0, 2)),
        "w2P": bf(w2.T.reshape(16, P, D).transpose(1, 0, 2)),
        "bqP": np.ascontiguousarray(
            np.asarray(inputs["bq"], f32).reshape(4, P).T),
        "b1P": np.ascontiguousarray(
            np.asarray(inputs["b1"], f32).reshape(16, P).T),
        "b22": bf(np.asarray(inputs["b2"], f32)[None, :]),
        "lnc": np.ascontiguousarray(lnc),
        "identf": np.eye(P, dtype=f32),
    }
    in_maps = []
    for i in range(NCORES):
        r0 = i * NQ
        m = dict(shared)
        m["hqT2"] = f8(_fold(np.ascontiguousarray(hT[:, r0:r0 + NQ])))
        m["hqP"] = np.ascontiguousarray(
            (h[r0:r0 + NQ, :] + bo2).reshape(4, P, D).transpose(1, 0, 2))
        m["maskP"] = np.ascontiguousarray(
            adjb_bf[r0:r0 + NQ, :].T.reshape(32, P, NQ).transpose(1, 0, 2))
        in_maps.append(m)
    return in_maps


def kernel(**inputs) -> np.ndarray:
    global _CACHED, LAST_EXEC_NS
    if _CACHED is None:
        _CACHED = _build()
    nc = _CACHED
    in_maps = _prep_inputs(inputs)
    kw = {}
    if TRACE:
        kw = dict(trace=True, tmpdir=TRACE_DIR)
    res = run_bass_kernel_spmd(nc, in_maps, list(range(NCORES)), **kw)
    LAST_EXEC_NS = res.exec_time_ns
    return np.concatenate([res.results[i]["out"] for i in range(NCORES)],
                          axis=0)
